# revision 25
# baseline (speedup 1.0000x reference)
"""Trainium2 Bass kernel for nn_InputLayer_57337813401914.

out[b, n, v] = sin(x[b, n] * freqs[v]), x: [64, 4096] f32,
freqs[v] = 10 ** (min(v, 127) / 127 * 4), v in [0, 256).

Sharding: batch dim (64) split across 8 NeuronCores; per core 32768 x
values, device output freq-major [256, 32768] (host transposes back).
The kernel is HBM-write-bound (33.5 MB/core at ~358 GB/s ~= 94 us), so
everything is organized to keep the store DMAs saturated.

Per-core pipeline, partition = frequency, 16 chunks of 2048 x values:
  PE    u[v, j]  = (freqs[v]/2pi) * x[j]   outer products -> PSUM. Run as
        K=6 bf16 matmuls on 3-way bf16 splits of x and f/2pi, keeping the
        six partial products >= |u|*2^-24: exact to ~2 ulp at full PE rate
        (fp32 matmul would be 8x slower and the bottleneck).
  w2    = fl(MAGIC - u) = MAGIC - rint(u)  magic-number rint; alternates
        ACT Identity(bias=MAGIC, scale=-1) / DVE tensor_scalar per chunk
        to balance engine load.
  DVE   r = (w2 - MAGIC) + u               one scalar_tensor_tensor; both
        subtractions exact (Sterbenz) => r = u - rint(u) in [-0.5, 0.5].
  ACT   s = Sin(2pi * r)                   table accurate on [-pi, pi]
  DMA   2 MB stores on the sync HWDGE ring (>=1 MiB => >=75% of peak).
Rows 127..255 all equal sin(1e4 * x): a tiny [128, 256] side pipeline
computes s127 first, then 16 broadcast DMAs on the scalar HWDGE ring
(4 front-loaded to cover the compute warmup, the rest one per chunk)
replicate it into rows 128..255. The x bf16 splits live on 6 partitions
(1-2 SBUF AXI ports, ~50 GB/s) so they stream in as 16 per-chunk slices
on the gpsimd ring instead of one 9 us blocking load.
Single f32-multiply range reduction: rel err vs reference ~4e-4
(tolerance 2e-2). ~104.5 us/core vs the 555 us staged baseline.
"""
import numpy as np
from contextlib import ExitStack

import concourse.bacc as bacc
import concourse.tile as tile
from concourse import mybir
from concourse.alu_op_type import AluOpType as A
from concourse.bass_utils import run_bass_kernel_spmd

P = 128            # SBUF/PSUM partitions; also the number of distinct freqs
NX = 32768         # x values per core
VLEAD = 128        # computed frequency rows (rows 128..255 replicate row 127)
V = 256            # total output rows
J = 2048           # x chunk per pipeline stage (free dim)
NCHUNK = NX // J   # 16
MMN = 512          # moving free dim per matmul (one PSUM bank of fp32)
NMM = J // MMN     # 4 matmuls per chunk
XROWS = 2          # x halves live on partitions 0 and 32 (matmul base rule)
XCOL = NX // XROWS
KSP = 6            # bf16 split products per outer product (see _in_maps)
NREP = 16          # replicate DMA count for rows 128..255
NCORES = 8
B, N = 64, 4096
B_PER_CORE = B // NCORES

MAGIC = float(np.float32(1.5 * 2**23))
TWO_PI = float(np.float32(2.0 * np.pi))

_BUILT = None


def _freqs_lead() -> np.ndarray:
    """First 128 freqs, bit-matching the reference (jnp f32 ops)."""
    try:
        import jax.numpy as jnp

        f = 10.0 ** (jnp.arange(VLEAD, dtype=jnp.float32) / (VLEAD - 1) * 4.0)
        return np.asarray(f, dtype=np.float32)
    except Exception:
        y = np.arange(VLEAD, dtype=np.float32) / np.float32(VLEAD - 1)
        y = y * np.float32(4.0)
        return np.power(np.float32(10.0), y, dtype=np.float32)


def _freqs_over_2pi() -> np.ndarray:
    return (
        _freqs_lead().astype(np.float64) / (2.0 * np.pi)
    ).astype(np.float32)


def _build(fp: np.ndarray):
    nc = bacc.Bacc(
        "TRN2", target_bir_lowering=False, debug=False, num_devices=NCORES
    )
    f32 = mybir.dt.float32
    bf16 = mybir.dt.bfloat16
    x6_in = nc.dram_tensor(
        "x6", [XROWS * KSP, XCOL], bf16, kind="ExternalInput"
    ).ap()
    x128_in = nc.dram_tensor("x128", [P, NX // P], f32, kind="ExternalInput").ap()
    f6_in = nc.dram_tensor(
        "f6", [XROWS * KSP, P], bf16, kind="ExternalInput"
    ).ap()
    out_t = nc.dram_tensor("out", [V, NX], f32, kind="ExternalOutput").ap()
    # out[v, p*256 + i] viewed partition-first: replicate-DMA destination
    # (the [128, 256] source spreads reads over all 16 SBUF AXI ports)
    out_v = out_t.rearrange("v (p i) -> p v i", p=P)

    fp127 = float(fp[VLEAD - 1])

    with tile.TileContext(nc) as tc:
        with ExitStack() as ctx:
            const = ctx.enter_context(tc.tile_pool(name="const", bufs=1))
            psum = ctx.enter_context(
                tc.tile_pool(name="psum", bufs=2, space="PSUM")
            )
            work = ctx.enter_context(tc.tile_pool(name="work", bufs=3))
            outp = ctx.enter_context(tc.tile_pool(name="outp", bufs=4))

            x6 = const.tile([32 + KSP, XCOL], bf16, tag="x6")
            x128 = const.tile([P, NX // P], f32, tag="x128")
            f6 = const.tile([32 + KSP, P], bf16, tag="f6")
            # x128 first on sync: it feeds the side pipeline whose sin
            # result unblocks the first (replicate) HBM write. The x6
            # halves land on 6 partitions (1-2 AXI ports, ~50 GB/s) so
            # they go on the scalar ring in parallel.
            nc.sync.dma_start(x128[:], x128_in[:])
            nc.sync.dma_start(f6[0:KSP, :], f6_in[0:KSP, :])
            nc.sync.dma_start(f6[32 : 32 + KSP, :], f6_in[KSP : 2 * KSP, :])
            # x6 lands on 6 partitions (1-2 AXI ports, ~50 GB/s): load it
            # as 16 per-chunk slices on the idle gpsimd ring, pipelined
            # ahead of each chunk's matmuls instead of one 9us blocker
            for cc in range(NCHUNK):
                srow = 32 * (cc // 8)
                irow = KSP * (cc // 8)
                ccol = (cc % 8) * J
                nc.gpsimd.dma_start(
                    x6[srow : srow + KSP, ccol : ccol + J],
                    x6_in[irow : irow + KSP, ccol : ccol + J],
                )
            # per-partition MAGIC column for the activation bias operand
            mgc = const.tile([P, 1], f32, tag="magic")
            nc.vector.memset(mgc[:], MAGIC)
            zro = const.tile([P, 1], f32, tag="zero")
            nc.vector.memset(zro[:], 0.0)

            # side pipeline: s127[p, i] = sin(1e4 * x[p*256+i]) feeds the
            # broadcast replicates for rows 128..255
            u7 = work.tile([P, NX // P], f32, tag="u7")
            nc.vector.tensor_scalar(u7[:], x128[:], fp127, None, A.mult)
            w7 = work.tile([P, NX // P], f32, tag="w7")
            nc.vector.tensor_scalar(
                w7[:], u7[:], -1.0, MAGIC, A.mult, A.add
            )
            r7 = work.tile([P, NX // P], f32, tag="r7")
            nc.vector.scalar_tensor_tensor(
                r7[:], w7[:], MAGIC, u7[:], A.subtract, A.add
            )
            s127 = const.tile([P, NX // P], f32, tag="s127")
            nc.scalar.activation(
                s127[:], r7[:], mybir.ActivationFunctionType.Sin,
                bias=zro[:], scale=TWO_PI,
            )

            rep_rows = (V - VLEAD) // NREP

            def replicate(rr):
                r0 = VLEAD + rr * rep_rows
                nc.scalar.dma_start(
                    out_v[:, r0 : r0 + rep_rows, :],
                    s127[:]
                    .unsqueeze(1)
                    .to_broadcast([P, rep_rows, NX // P]),
                )

            # front-load replicates as soon as s127 lands: they are the
            # only HBM writes available while the main pipeline warms up
            NFRONT = 4
            for rr in range(NFRONT):
                replicate(rr)

            # software-pipelined main loop: ACT queue order is
            # w2(0), w2(1), sin(0), w2(2), sin(1), ... so the per-chunk
            # ACT->DVE->ACT chain overlaps across chunks.
            prev = None  # (r_tile, u_tile, c) pending sin+store
            for c in range(NCHUNK + 1):
                if c < NCHUNK:
                    u_t = psum.tile([P, J], f32, tag="u")
                    for m in range(NMM):
                        g = c * NMM + m
                        row = 32 * (g // (XCOL // MMN))
                        col = (g % (XCOL // MMN)) * MMN
                        nc.tensor.matmul(
                            u_t[:, m * MMN : (m + 1) * MMN],
                            f6[row : row + KSP, :],
                            x6[row : row + KSP, col : col + MMN],
                            start=True,
                            stop=True,
                        )
                    w2_t = work.tile([P, J], f32, tag="w2")
                    if c % 2 == 0:
                        # alternate w2 between ACT and DVE so neither sits
                        # near the DMA-saturated critical path
                        nc.scalar.activation(
                            w2_t[:], u_t[:],
                            mybir.ActivationFunctionType.Identity,
                            bias=mgc[:], scale=-1.0,
                        )
                    else:
                        nc.vector.tensor_scalar(
                            w2_t[:], u_t[:], -1.0, MAGIC, A.mult, A.add
                        )
                    r_t = work.tile([P, J], f32, tag="r")
                    nc.vector.scalar_tensor_tensor(
                        r_t[:], w2_t[:], MAGIC, u_t[:], A.subtract, A.add
                    )
                    prev_next = (r_t, c)
                else:
                    prev_next = None

                if prev is not None:
                    r_p, cp = prev
                    if cp % 2 == 0:
                        s2 = outp.tile([P, 2, J], f32, tag="s")
                    nc.scalar.activation(
                        s2[:, cp % 2, :], r_p[:],
                        mybir.ActivationFunctionType.Sin,
                        bias=zro[:], scale=TWO_PI,
                    )
                    if cp % 2 == 1:
                        # 2 MB store: >=1 MiB transfers run at >=75% of peak
                        nc.sync.dma_start(
                            out_t[0:VLEAD, (cp - 1) * J : (cp + 1) * J],
                            s2[:],
                        )
                    # remaining replicates: one after every chunk's sin on
                    # the scalar HWDGE ring, so replicate bytes always sit
                    # queued behind at most one transfer (short ACT stalls)
                    if NFRONT + cp < NREP:
                        replicate(NFRONT + cp)
                prev = prev_next

    nc.compile()
    return nc


def _split3(a: np.ndarray):
    """Exact-ish 3-way bf16 split: h + m + l == a to within ~2^-25 rel."""
    import ml_dtypes

    bf = ml_dtypes.bfloat16
    h = a.astype(bf)
    m = (a - h.astype(np.float32)).astype(bf)
    l = (a - h.astype(np.float32) - m.astype(np.float32)).astype(bf)
    return h, m, l


def _in_maps(x: np.ndarray):
    """Per-core input dict. The 6 bf16 K-rows pair as
    (fh,xh) (fh,xm) (fh,xl) (fm,xh) (fm,xm) (fl,xh) — every partial
    product of magnitude >= |u| * 2^-24."""
    import ml_dtypes

    fp = _freqs_over_2pi()
    fh, fm, fl = _split3(fp)
    f_rows = np.stack([fh, fh, fh, fm, fm, fl])  # [KSP, 128] bf16
    f6 = np.ascontiguousarray(
        np.tile(f_rows, (XROWS, 1)).astype(ml_dtypes.bfloat16)
    )
    in_maps = []
    for c in range(NCORES):
        xs = x[c * B_PER_CORE : (c + 1) * B_PER_CORE].reshape(-1)
        xh, xm, xl = _split3(xs.reshape(XROWS, XCOL))
        x6 = np.ascontiguousarray(
            np.stack([xh[0], xm[0], xl[0], xh[0], xm[0], xh[0],
                      xh[1], xm[1], xl[1], xh[1], xm[1], xh[1]])
        )
        in_maps.append(
            {
                "x6": x6,
                "x128": np.ascontiguousarray(xs.reshape(P, NX // P)),
                "f6": f6,
            }
        )
    return in_maps


def kernel(x, vector_size):
    global _BUILT
    x = np.asarray(x, dtype=np.float32)
    assert x.shape == (B, N), x.shape
    assert int(vector_size) == V, vector_size

    if _BUILT is None:
        _BUILT = _build(_freqs_over_2pi())
    nc = _BUILT

    res = run_bass_kernel_spmd(nc, _in_maps(x), list(range(NCORES)))

    out = np.empty((B, N, V), dtype=np.float32)
    for c in range(NCORES):
        oc = res.results[c]["out"]  # [256, 32768] freq-major
        out[c * B_PER_CORE : (c + 1) * B_PER_CORE] = np.ascontiguousarray(
            oc.T
        ).reshape(B_PER_CORE, N, V)
    return out
